# revision 3
# baseline (speedup 1.0000x reference)
"""Trainium2 Bass kernel for the NMS-detection problem.

Contract: kernel(**inputs) takes the FULL inputs
    tmap_raw  (B,4,64,64) f32, logit_raw (B,1,64,64) f32,
    n_objects_max (int), topk_only (int)
and returns the reference's output tuple
    (prob_few, bx_few, by_few, bw_few, bh_few), each (n_objects_max, B) f32.

Sharding: data-parallel over the batch dim. Core c computes batch element
c % B entirely on-chip (greedy NMS is sequential per batch element); the
host gathers the per-core (k,5) records from cores 0..B-1.

Device algorithm (per core): boxes live in a (128,32) SBUF layout
(box i = p*32 + j, i = ix*64 + iy). Greedy NMS picks argmax(prob*possible)
k times; each pick is recorded immediately — the picks come out in
descending-prob order, which equals the reference's top_k(masked_prob)
order (the reference's NMS always finds k valid boxes for these inputs,
verified numerically). Suppression rows are computed on the fly from the
chosen box's geometry instead of materializing the (n,n) overlap matrix.
"""

from contextlib import ExitStack

import numpy as np

import concourse.bass as bass
import concourse.bacc as bacc
import concourse.tile as tile
import concourse.mybir as mybir
from concourse.bass_utils import run_bass_kernel_spmd

F32 = mybir.dt.float32
ALU = mybir.AluOpType
ACTF = mybir.ActivationFunctionType

N = 4096
P = 128
J = 32  # free cols per partition; box index i = p*J + j
N_CORES = 8


def _make_consts():
    i = np.arange(N, dtype=np.float32)
    return {
        "c_iota_m": (i - N).reshape(P, J).astype(np.float32),
        "c_ixg": np.floor(i / 64).reshape(P, J).astype(np.float32),
        "c_iyg": np.mod(i, 64).reshape(P, J).astype(np.float32),
        "c_ident": np.eye(P, dtype=np.float32),
        "c_ones": np.ones((1, P), dtype=np.float32),
    }


def _build(nobj, topk_only):
    nc = bacc.Bacc("TRN2", target_bir_lowering=False, debug=False,
                   num_devices=N_CORES)

    traw = nc.dram_tensor("traw", [4, P, J], F32, kind="ExternalInput").ap()
    lraw = nc.dram_tensor("lraw", [P, J], F32, kind="ExternalInput").ap()
    c_iota = nc.dram_tensor("c_iota_m", [P, J], F32, kind="ExternalInput").ap()
    c_ixg = nc.dram_tensor("c_ixg", [P, J], F32, kind="ExternalInput").ap()
    c_iyg = nc.dram_tensor("c_iyg", [P, J], F32, kind="ExternalInput").ap()
    c_ident = nc.dram_tensor("c_ident", [P, P], F32, kind="ExternalInput").ap()
    c_ones = nc.dram_tensor("c_ones", [1, P], F32, kind="ExternalInput").ap()
    nrec = max(256, ((nobj * 5 + 31) // 32) * 32)
    out_d = nc.dram_tensor("outrec", [1, nrec], F32, kind="ExternalOutput").ap()

    with tile.TileContext(nc) as tc, ExitStack() as ctx:
        _body(ctx, tc, traw, lraw, c_iota, c_ixg, c_iyg, c_ident, c_ones,
              out_d, nrec, nobj, topk_only)
    nc.compile()
    return nc


def _body(ctx, tc, traw, lraw, c_iota, c_ixg, c_iyg, c_ident, c_ones, out_d,
          nrec, nobj, topk_only):
    nc = tc.nc
    v = nc.vector
    s = nc.scalar
    t = nc.tensor

    cpool = ctx.enter_context(tc.tile_pool(name="consts", bufs=1))
    ppool = ctx.enter_context(tc.tile_pool(name="persist", bufs=1))
    wpool = ctx.enter_context(tc.tile_pool(name="work", bufs=2))
    qpool = ctx.enter_context(tc.tile_pool(name="psum", bufs=1, space="PSUM"))
    q2pool = ctx.enter_context(tc.tile_pool(name="psum2", bufs=1, space="PSUM"))

    # ---- load constants & inputs -------------------------------------------
    iota_m = cpool.tile([P, J], F32, tag="iota")
    nc.sync.dma_start(iota_m[:], c_iota)
    ixg = cpool.tile([P, J], F32, tag="ixg")
    nc.sync.dma_start(ixg[:], c_ixg)
    iyg = cpool.tile([P, J], F32, tag="iyg")
    nc.sync.dma_start(iyg[:], c_iyg)
    ident = cpool.tile([P, P], F32, tag="ident")
    nc.sync.dma_start(ident[:], c_ident)
    ones_row = cpool.tile([1, P], F32, tag="ones")
    nc.sync.dma_start(ones_row[:], c_ones)

    tin = ppool.tile([P, 4 * J], F32, tag="tin")
    for c in range(4):
        nc.sync.dma_start(tin[:, c * J:(c + 1) * J], traw[c])
    lin = ppool.tile([P, J], F32, tag="lin")
    nc.sync.dma_start(lin[:], lraw)

    # ---- preprocessing ------------------------------------------------------
    # allcat column blocks (J=32 wide):
    # 0:x1 1:x3 2:y1 3:y3 4:area 5:prob 6:bx 7:by 8:bw 9:bh 10:cand
    allcat = ppool.tile([P, 11 * J], F32, tag="allcat")
    blk = lambda k: allcat[:, k * J:(k + 1) * J]
    x1_sl, x3_sl, y1_sl, y3_sl = blk(0), blk(1), blk(2), blk(3)
    area_sl, prob_sl = blk(4), blk(5)
    bx_sl, by_sl, bw_sl, bh_sl = blk(6), blk(7), blk(8), blk(9)
    cand_sl = blk(10)

    tx = wpool.tile([P, J], F32, tag="tx")
    ty = wpool.tile([P, J], F32, tag="ty")
    tw = wpool.tile([P, J], F32, tag="tw")
    th = wpool.tile([P, J], F32, tag="th")
    s.activation(tx[:], tin[:, 0 * J:1 * J], ACTF.Sigmoid)
    s.activation(ty[:], tin[:, 1 * J:2 * J], ACTF.Sigmoid)
    s.activation(tw[:], tin[:, 2 * J:3 * J], ACTF.Sigmoid)
    s.activation(th[:], tin[:, 3 * J:4 * J], ACTF.Sigmoid)
    s.activation(prob_sl, lin[:], ACTF.Sigmoid)

    # bx = 8*(ix+tx), by = 8*(iy+ty)   (== 512*(ix+tx)/64 exactly)
    v.tensor_tensor(bx_sl, ixg[:], tx[:], op=ALU.add)
    v.tensor_scalar(bx_sl, bx_sl, 8.0, None, op0=ALU.mult)
    v.tensor_tensor(by_sl, iyg[:], ty[:], op=ALU.add)
    v.tensor_scalar(by_sl, by_sl, 8.0, None, op0=ALU.mult)
    # bw = 10 + 30*tw ; bh = 10 + 30*th
    v.tensor_scalar(bw_sl, tw[:], 30.0, 10.0, op0=ALU.mult, op1=ALU.add)
    v.tensor_scalar(bh_sl, th[:], 30.0, 10.0, op0=ALU.mult, op1=ALU.add)
    # x1 = bx - 0.5*bw etc (same rounding as reference)
    v.scalar_tensor_tensor(x1_sl, bw_sl, -0.5, bx_sl, op0=ALU.mult, op1=ALU.add)
    v.scalar_tensor_tensor(x3_sl, bw_sl, 0.5, bx_sl, op0=ALU.mult, op1=ALU.add)
    v.scalar_tensor_tensor(y1_sl, bh_sl, -0.5, by_sl, op0=ALU.mult, op1=ALU.add)
    v.scalar_tensor_tensor(y3_sl, bh_sl, 0.5, by_sl, op0=ALU.mult, op1=ALU.add)
    v.tensor_tensor(area_sl, bw_sl, bh_sl, op=ALU.mult)

    possible = ppool.tile([P, J], F32, tag="possible")
    v.memset(possible[:], 1.0)

    outrec = ppool.tile([1, nrec], F32, tag="outrec")
    v.memset(outrec[:], 0.0)

    # ---- greedy NMS loop ----------------------------------------------------
    for l in range(nobj):
        # score = prob*possible; per-partition max
        # (tensor_tensor_reduce would fuse these but crashes TRN2 HW)
        score = wpool.tile([P, J], F32, tag="score")
        pmax = wpool.tile([P, 1], F32, tag="pmax")
        v.tensor_tensor(score[:], prob_sl, possible[:], op=ALU.mult)
        v.tensor_reduce(pmax[:], score[:], axis=mybir.AxisListType.X, op=ALU.max)

        # global max via PE transpose + free-dim reduce
        ps_t = q2pool.tile([1, P], F32, tag="ps_t")
        t.transpose(ps_t[:], pmax[:], ident[:])
        gmax = wpool.tile([1, 1], F32, tag="gmax")
        v.tensor_reduce(gmax[:], ps_t[:], axis=mybir.AxisListType.X, op=ALU.max)

        # broadcast gmax to all partitions; ge-mask; first index of max
        ps_b = q2pool.tile([P, 1], F32, tag="ps_b")
        t.matmul(ps_b[:], ones_row[:], gmax[:])
        ge = wpool.tile([P, J], F32, tag="ge")
        v.tensor_scalar(ge[:], score[:], ps_b[:, 0:1], None, op0=ALU.is_ge)
        imin = wpool.tile([P, 1], F32, tag="imin")
        v.tensor_tensor(cand_sl, ge[:], iota_m[:], op=ALU.mult)
        v.tensor_reduce(imin[:], cand_sl, axis=mybir.AxisListType.X, op=ALU.min)
        ps_t2 = q2pool.tile([1, P], F32, tag="ps_t2")
        t.transpose(ps_t2[:], imin[:], ident[:])
        gidx = wpool.tile([1, 1], F32, tag="gidx")
        v.tensor_reduce(gidx[:], ps_t2[:], axis=mybir.AxisListType.X, op=ALU.min)

        # partition-onehot of the winner; extract its 10 stats via matmul
        ps_c = q2pool.tile([P, 1], F32, tag="ps_c")
        t.matmul(ps_c[:], ones_row[:], gidx[:])
        ohp = wpool.tile([P, 1], F32, tag="ohp")
        v.tensor_scalar(ohp[:], imin[:], ps_c[:, 0:1], None, op0=ALU.is_equal)
        ps_d = qpool.tile([1, 11 * J], F32, tag="ps_d")
        t.matmul(ps_d[:], ohp[:], allcat[:])
        eqj = wpool.tile([1, J], F32, tag="eqj")
        v.tensor_scalar(eqj[:], ps_d[:, 10 * J:11 * J], gidx[:], None,
                        op0=ALU.is_equal)
        prod = wpool.tile([1, 10 * J], F32, tag="prod")
        eqj_b = bass.AP(eqj.tensor, eqj[:].offset,
                        [list(eqj[:].ap[0]), [0, 10], [1, J]])
        v.tensor_tensor(prod[:].rearrange("a (m j) -> a m j", j=J),
                        ps_d[:, 0:10 * J].rearrange("a (m j) -> a m j", j=J),
                        eqj_b, op=ALU.mult)
        vals = wpool.tile([1, 10], F32, tag="vals")
        v.tensor_reduce(vals[:], prod[:].rearrange("a (m j) -> a m j", j=J),
                        axis=mybir.AxisListType.X, op=ALU.add)

        # record [prob,bx,by,bw,bh] at slot l (off critical path, on ACT)
        s.copy(outrec[:, l * 5:(l + 1) * 5], vals[:, 5:10])

        if topk_only:
            # plain top-k: only remove the chosen box
            oh_full = wpool.tile([P, J], F32, tag="ohfull")
            v.tensor_scalar(oh_full[:], cand_sl, ps_c[:, 0:1], None,
                            op0=ALU.is_equal)
            v.scalar_tensor_tensor(possible[:], oh_full[:], -1.0, possible[:],
                                   op0=ALU.mult, op1=ALU.add)
            continue

        # suppression row of the winner, applied to `possible`
        ps_h = qpool.tile([P, 5], F32, tag="ps_h")
        t.matmul(ps_h[:], ones_row[:], vals[:, 0:5])
        t_a = wpool.tile([P, J], F32, tag="t_a")
        v.tensor_scalar(t_a[:], x1_sl, ps_h[:, 0:1], None, op0=ALU.max)
        t_w = wpool.tile([P, J], F32, tag="t_w")
        v.scalar_tensor_tensor(t_w[:], x3_sl, ps_h[:, 1:2], t_a[:],
                               op0=ALU.min, op1=ALU.subtract)
        v.tensor_scalar(t_w[:], t_w[:], 0.0, None, op0=ALU.max)
        t_b = wpool.tile([P, J], F32, tag="t_b")
        v.tensor_scalar(t_b[:], y1_sl, ps_h[:, 2:3], None, op0=ALU.max)
        t_h = wpool.tile([P, J], F32, tag="t_h")
        v.scalar_tensor_tensor(t_h[:], y3_sl, ps_h[:, 3:4], t_b[:],
                               op0=ALU.min, op1=ALU.subtract)
        v.tensor_scalar(t_h[:], t_h[:], 0.0, None, op0=ALU.max)
        t_i = wpool.tile([P, J], F32, tag="t_i")
        v.tensor_tensor(t_i[:], t_w[:], t_h[:], op=ALU.mult)
        t_m = wpool.tile([P, J], F32, tag="t_m")
        v.tensor_scalar(t_m[:], area_sl, ps_h[:, 4:5], None, op0=ALU.min)
        t_z = wpool.tile([P, J], F32, tag="t_z")
        # z = 0.3*min_area - inter ; keep box iff z >= 0
        v.scalar_tensor_tensor(t_z[:], t_m[:], 0.3, t_i[:],
                               op0=ALU.mult, op1=ALU.subtract)
        v.scalar_tensor_tensor(possible[:], t_z[:], 0.0, possible[:],
                               op0=ALU.is_ge, op1=ALU.mult)

    nc.sync.dma_start(out_d, outrec[:])


_CACHE = {}


def _get_program(nobj, topk_only):
    key = (nobj, topk_only)
    if key not in _CACHE:
        _CACHE[key] = _build(nobj, topk_only)
    return _CACHE[key]


def run_on_device(tmap_raw, logit_raw, n_objects_max, topk_only,
                  trace=False, tmpdir=None):
    """Shard over cores, run, and return (outputs_tuple, BassKernelResults)."""
    nobj = int(n_objects_max)
    tk = int(np.asarray(topk_only))
    tmap = np.ascontiguousarray(np.asarray(tmap_raw, dtype=np.float32))
    logit = np.ascontiguousarray(np.asarray(logit_raw, dtype=np.float32))
    B = tmap.shape[0]

    nc = _get_program(nobj, tk)
    consts = _make_consts()
    in_maps = []
    for c in range(N_CORES):
        b = c % B
        in_maps.append({
            "traw": tmap[b].reshape(4, P, J),
            "lraw": logit[b, 0].reshape(P, J),
            **consts,
        })
    kw = {}
    if trace:
        kw = dict(trace=True, tmpdir=tmpdir)
    bres = run_bass_kernel_spmd(nc, in_maps, list(range(N_CORES)), **kw)
    res = bres.results

    K = nobj
    outs = [np.zeros((K, B), np.float32) for _ in range(5)]
    for b in range(B):
        rec = np.asarray(res[b]["outrec"]).reshape(-1)[:K * 5].reshape(K, 5)
        for m in range(5):
            outs[m][:, b] = rec[:, m]
    return tuple(outs), bres


def kernel(tmap_raw, logit_raw, n_objects_max, topk_only):
    outs, _ = run_on_device(tmap_raw, logit_raw, n_objects_max, topk_only)
    return outs
